# revision 46
# baseline (speedup 1.0000x reference)
"""Multi-head causal attention (B=2, S=2048, D=1024, H=16) on 8 TRN2 cores.

Sharding (Megatron-style): core c handles batch b = c//4, head-group
g = c%4 (4 heads, d' slice of 256). Each core computes its partial
out = ctx_g @ Wo[gslice] (no bias) in fp16; host sums the 4 partials
per batch in fp32 and adds the bias.

Device kernel dataflow (fp16 matmul operands, fp32 PSUM accumulation):
  qT [d', S] and v via PE projections; kT split-evicted into per-head
  ZERO-PADDED [128, S] buffers so each scores matmul contracts a full
  128 partitions (padding rows x other head's q = 0) and lowers as a
  normal full-array matmul — the previous K=64 row-group pairs
  serialized every LDWEIGHTS against stream drain and kept dropping
  the HAM clock gate; the padded form runs the whole mid-kernel as a
  clean 215ns/matmul conveyor ->
  scores sT[k, q] per (head, k-tile, q-block) -> exp on ACT
  (psum->sbuf, fused 1/sqrt(hd) scale) -> causal triangle masking via
  GPSIMD affine_select in-place; fully masked regions are simply
  skipped by partial-width matmuls -> ctxT + softmax denominators
  accumulated on PE (ones columns interleaved in the v operand) ->
  1/denom = exp(-ln d) on ACT -> normalize on DVE (shifted-in1 mul)
  -> out projection on PE.

Schedule: single j-loop interleaving both head pairs per q-block so the
ACT exp load is spread across the whole kernel instead of bunching in a
second phase (which measured ACT-bound + power-throttled). k-tiles are
processed in groups of 2 (scores x2 -> exp x2 -> ctx x4). Out-proj
chunks ride at the end of each j so the output DMA streams throughout.
Startup: only the j=0 critical set (x block 0 on sync, wq0/wk0/wv on
scalar) competes for early HBM bandwidth; wq1/wk1 gate behind the q0
eviction on gpsimd, x1-3/wo issue behind ACT-queue progress; 72 PE
warmup matmuls bridge the DMA fill so the HAM clock gate stays hot.
Tail: the final attention skips its normalize; the epilogue runs a
monolithic ln/exp, 256-col mul chunks feeding the pair-1 out-proj
matmuls, and post-loop evictions with the last chunk's halves DMAing
on both queues.
"""

import numpy as np

import concourse.bass as bass
import concourse.mybir as mybir
import concourse.tile as tile
from concourse import bacc, bass_utils
from concourse.hw_specs import get_activation_tables

F32 = mybir.dt.float32
F16 = mybir.dt.float16
EXP = mybir.ActivationFunctionType.Exp
LN = mybir.ActivationFunctionType.Ln

B, S, D, H, HD = 2, 2048, 1024, 16, 64
NHL = 4          # local heads per core
DC = NHL * HD    # 256 local d'
NDT = D // 128   # 8 contraction tiles for projections
KT = 128         # k tile
NKT = S // KT    # 16
QB = 512         # q block
NQB = S // QB    # 4
SCALE = 1.0 / np.sqrt(HD)

# va free-layout per k-tile m: [v0 | J | v1 | v2 | J | v3], J = ones(64)
# (matmul weight APs allow only one free dim, so the ones blocks are
# interleaved to make every head a contiguous 128-col slice).
# Head h reads 128 cols at VA_OFF[h]; even heads are [v|J] (ctx psum rows
# 0:64, denom 64:128), odd heads [J|v] (denom 0:64, ctx 64:128).
VA_W = 384
VA_OFF = [0, 64, 192, 256]


def _pin_act_table(arch):
    """Steer Bacc's activation-table chooser to the one set containing both
    exp and ln so ACT never thrashes ACT_TABLE_LOADs (1.28us each)."""
    tabs = get_activation_tables(arch)
    keep = "natural_log_exp_and_others"
    if keep not in tabs:
        return
    for name, funcs in tabs.items():
        if name != keep:
            funcs.discard(EXP)
            funcs.discard(LN)


def build_nc():
    nc = bacc.Bacc("TRN2", target_bir_lowering=False, debug=False)
    xT = nc.dram_tensor("xT", [128, NDT, S], F16, kind="ExternalInput")
    wq = nc.dram_tensor("wq", [128, 2, NDT, 128], F16, kind="ExternalInput")
    wk = nc.dram_tensor("wk", [128, 2, NDT, 128], F16, kind="ExternalInput")
    wv = nc.dram_tensor("wv", [128, NDT, DC], F16, kind="ExternalInput")
    wo = nc.dram_tensor("wo", [128, 2, D], F16, kind="ExternalInput")
    out = nc.dram_tensor("out_p", [S, D], F16, kind="ExternalOutput")

    with tile.TileContext(nc) as tc:
        with (
            tc.tile_pool(name="xp", bufs=1) as xp,
            tc.tile_pool(name="wp", bufs=1) as wp,
            tc.tile_pool(name="qk", bufs=1) as qkp,
            tc.tile_pool(name="vap", bufs=1) as vap,
            tc.tile_pool(name="cnp", bufs=1) as cnp,
            tc.tile_pool(name="et", bufs=8) as etp,
            tc.tile_pool(name="rcp", bufs=4) as rcp,
            tc.tile_pool(name="ob", bufs=4) as obp,
            tc.tile_pool(name="pp", bufs=2, space="PSUM") as pp,
            tc.tile_pool(name="sp", bufs=2, space="PSUM") as sp,
            tc.tile_pool(name="cp", bufs=1, space="PSUM") as cp,
        ):
            # ---- loads: pair-0 weight halves first so the j=0 q/k chains
            # can start as soon as x block 0 lands; x column-blocked on the
            # sync queue; wv/wo ride the vector engine's queue.
            wq_sb = wp.tile([128, 2, NDT, 128], F16, tag="wq")
            wk_sb = wp.tile([128, 2, NDT, 128], F16, tag="wk")
            wv_sb = wp.tile([128, NDT, DC], F16, tag="wv")
            wo_sb = wp.tile([128, 2, D], F16, tag="wo")
            x_sb = xp.tile([128, NDT, S], F16)
            # Startup DMA: ONLY what j=0 needs competes for early HBM
            # bandwidth — x block 0 (sync queue) + wq0/wk0/wv (scalar
            # queue) = 3MiB, ready ~15us.  Everything else (wq1/wk1, x
            # blocks 1-3, wo) is DEFERRED: their dma_starts are emitted on
            # the VECTOR engine after specific early evictions, so the
            # in-order DVE queue naturally delays the issue until the j=0
            # pipeline is rolling and the early transfers have drained.
            nc.scalar.dma_start(wq_sb[:, 0], wq.ap()[:, 0])
            nc.scalar.dma_start(wk_sb[:, 0], wk.ap()[:, 0])
            # x block 0 in t-quarters on the sync queue (alone — a queue
            # round-robins its pending descriptors, so rivals delay the
            # critical first quarters); the j=0 chains' per-matmul deps
            # let the PE start on quarter 1 instead of the whole MiB.
            for tq in range(0, NDT, 2):
                nc.sync.dma_start(x_sb[:, tq:tq + 2, 0:QB],
                                  xT.ap()[:, tq:tq + 2, 0:QB])
            xt_sb = [x_sb[:, t, :] for t in range(NDT)]

            # ---- constants; warmup matmuls ramp the HAM clock gate while
            # the input DMAs stream. wu needs only a fast gpsimd memset.
            wu = wp.tile([128, 128], F16, tag="wu")
            # DVE memset: the gpsimd queue is busy with SWDGE descriptor
            # generation right after its preamble, which would delay the
            # PE warmup by ~2.5us.
            nc.vector.memset(wu[:], 1.0)
            # wv rides scalar behind wq0/wk0; wq1/wk1 are deferred behind
            # a gpsimd gate (emitted after the q0 chain below).
            nc.scalar.dma_start(wv_sb[:], wv.ap())
            # warmups bridge the DMA-wait gap so the HAM clock gate never
            # drops back to k=4 right as the j=0 chains start (measured:
            # 48 warmups end 12.2us, first chain matmul 16.6us, HAM
            # throttled 17.0-23.9us at half rate).
            wups = pp.tile([128, 128], F32, tag="pp", name="wups")
            for _ in range(80):
                nc.tensor.matmul(wups[:], wu[:], wu[:], start=True, stop=True)
            # zero weights for the epilogue's keep-alive matmuls: a matmul
            # with an all-zero stationary adds exact +0.0 to its psum
            # accumulation, so it is a numerical no-op that keeps the PE
            # streaming (HAM clock gate) through the final normalize.
            wz = wp.tile([128, 128], F16, tag="wz")
            nc.gpsimd.memset(wz[:], 0.0)
            tri = wp.tile([128, 128], F16, tag="tri")
            nc.gpsimd.memset(tri[:], 1.0)
            # tri[k, q] = 1 if q >= k else 0
            nc.gpsimd.affine_select(
                out=tri[:], in_=tri[:], compare_op=mybir.AluOpType.is_ge,
                fill=0.0, base=0, pattern=[[1, 128]], channel_multiplier=-1)
            # [128, NKT, 2, 192] view of the [v0|J|v1|v2|J|v3] layout: each
            # head's 128-col window lives inside one 192 block, and the
            # (v0,v2)/(v1,v3) eviction pairs and both J strips become single
            # strided APs.
            va = vap.tile([128, NKT, 2, VA_W // 2], F16)
            # only the ones-strips J; v strips are written by proj_v
            nc.gpsimd.memset(va[:, :, :, 64:128], 1.0)
            warmup = wp.tile([1, 8], F32, tag="wuact")
            nc.vector.memset(warmup[:], 1.0)
            nc.scalar.activation(warmup[:], warmup[:], EXP, scale=0.001)

            # ---- projections ----
            qT = [qkp.tile([128, S], F16, tag=f"q{hp}", name=f"q{hp}")
                  for hp in range(2)]
            # kT stored PADDED per local head: kTp[hp][lh] is [128, S] with
            # the real kT rows in 64*lh:64*(lh+1) and ZEROS in the other
            # half.  The scores matmul then contracts a full 128 partitions
            # (zero rows x other head's q = 0), so it lowers as a normal
            # full-array matmul whose LDWEIGHTS pipelines behind the
            # previous stream — the 64-row row-group form serialized every
            # weight load against stream drain (~400ns/pair measured).
            kTp = [[qkp.tile([128, S], F16, tag=f"k{hp}{lh}",
                             name=f"k{hp}{lh}") for lh in range(2)]
                   for hp in range(2)]
            ctxn = [cnp.tile([128, S], F16, tag=f"c{t}", name=f"c{t}")
                    for t in range(2)]
            for hp in range(2):
                nc.gpsimd.memset(kTp[hp][0][64:128, :], 0.0)
                nc.gpsimd.memset(kTp[hp][1][0:64, :], 0.0)

            def proj_chain(hp, dst, w_sb, j):
                ps = pp.tile([128, QB], F32, tag="pp")
                for t in range(NDT):
                    nc.tensor.matmul(
                        ps[:], w_sb[:, hp, t, :],
                        xt_sb[t][:, QB * j:QB * (j + 1)],
                        start=(t == 0), stop=(t == NDT - 1))
                jsl = slice(QB * j, QB * (j + 1))
                if isinstance(dst, list):  # k: split-evict into padded lh
                    nc.vector.tensor_copy(dst[0][0:64, jsl], ps[0:64, :])
                    nc.vector.tensor_copy(dst[1][64:128, jsl], ps[64:128, :])
                else:
                    nc.vector.tensor_copy(dst[:, jsl], ps[:])

            def proj_v(lo, hi):
                for m in range(lo, hi):
                    ps = pp.tile([128, 2, DC // 2], F32, tag="pp")
                    for t in range(NDT):
                        nc.tensor.matmul(
                            ps[:, :, :], xt_sb[t][:, 128 * m:128 * (m + 1)],
                            wv_sb[:, t, :], start=(t == 0), stop=(t == NDT - 1))
                    # 2 strided copies: (v0,v2) -> block starts, (v1,v3) ->
                    # block offsets 128.
                    nc.vector.tensor_copy(va[:, m, :, 0:64], ps[:, :, 0:64])
                    nc.vector.tensor_copy(va[:, m, :, 128:192],
                                          ps[:, :, 64:128])

            def attention_j(hp, j, fillers=(), normalize=True):
                cpb = cp.tile([128, 2, QB], F32, tag="cp")  # heads 2hp, 2hp+1
                cpe, cpo = cpb[:, 0], cpb[:, 1]
                nkt_j = 4 * j + 4
                # filler PE units (proj chains / out chunks) are emitted
                # between the scores and ctx matmuls of evenly spread
                # k-tile GROUPS: within attention the ACT exp (~1.15us/tile)
                # outruns the PE (~0.86us/tile), so without filler the PE
                # starves; the first unit also covers the ctx-psum WAR on
                # the previous attention's normalize.
                nf = len(fillers)
                fi = 0
                # diagonal k-tiles first: their exp+affine_select chains
                # are absorbed during pipeline fill and the attention ENDS
                # on full tiles whose ctx trails only a plain exp — the
                # final ctx otherwise stalls the PE queue head ~3.5us
                # behind the ACT backlog + Pool hop.  Tile 4j has c0=0 so
                # the psum accumulation still opens full-width.
                # k-tiles are processed in GROUPS of 2 (scores x2, exp x2,
                # ctx x4): every entry/exit of the PE's 64-row-group mode
                # serializes LDWEIGHTS against the previous stream's drain
                # (~200ns/pair measured); pairing tiles halves the
                # switches.  sp pool bufs=2 holds both score tiles.
                order = list(range(4 * j, nkt_j)) + list(range(4 * j))
                groups = [order[g:g + 2] for g in range(0, nkt_j, 2)]
                ng = len(groups)
                due = sorted(k * ng // nf for k in range(nf)) if nf else []
                for g_i, grp in enumerate(groups):
                    sps = []
                    for i in grp:
                        c0 = max(0, 128 * (i - 4 * j))
                        spt = sp.tile([128, 2, QB], F32, tag="sp")
                        for lh in range(2):
                            nc.tensor.matmul(
                                spt[:, lh, c0:QB],
                                kTp[hp][lh][:, 128 * i:128 * (i + 1)],
                                qT[hp][:, QB * j + c0:QB * (j + 1)],
                                start=True, stop=True)
                        sps.append((i, c0, spt))
                    while fi < nf and due[fi] == g_i:
                        fillers[fi]()
                        fi += 1
                    ets = []
                    for i, c0, spt in sps:
                        et = etp.tile([128, 2, QB], F16, tag="et")
                        nc.scalar.activation(et[:, :, c0:QB], spt[:, :, c0:QB],
                                             EXP, scale=float(SCALE))
                        if i >= 4 * j:  # diagonal: triangle mask in place
                            sl = et[:, :, c0:c0 + 128]
                            nc.gpsimd.affine_select(
                                out=sl, in_=sl,
                                compare_op=mybir.AluOpType.is_ge, fill=0.0,
                                base=0, pattern=[[0, 2], [1, 128]],
                                channel_multiplier=-1)
                        ets.append((i, c0, et))
                    for t_i, (i, c0, et) in enumerate(ets):
                        k_i = 2 * g_i + t_i
                        for lh, cpt in ((0, cpe), (1, cpo)):
                            off = VA_OFF[2 * hp + lh]
                            blk, o = off // 192, off % 192
                            nc.tensor.matmul(
                                cpt[:, c0:QB],
                                va[:, i, blk, o:o + 128],
                                et[:, lh, c0:QB],
                                start=(k_i == 0), stop=(k_i == nkt_j - 1))
                assert fi == nf, (fi, nf, hp, j)
                if not normalize:
                    i, c0, et = ets[-1]
                    return cpb, et, c0
                # normalize; even heads [v|J]: ctx rows 0:64, denom 64:128;
                # odd heads [J|v]: denom 0:64, ctx 64:128.  1/denom =
                # exp(-ln(denom)) on ACT over the whole [128, 2, QB] block
                # (one ln + one exp for both heads; ctx-row lanes produce
                # junk that is never read; DVE reciprocal measured 3.3us
                # per tile and gpsimd divide doesn't compile), then
                # shifted-in1 DVE muls (verified exact on HW).
                t1 = rcp.tile([128, 2, QB], F32, tag="t1")
                rc = rcp.tile([128, 2, QB], F32, tag="rc")
                nc.scalar.activation(t1[:], cpb[:], LN)
                nc.scalar.activation(rc[:], t1[:], EXP, scale=-1.0)
                for lh, cpt in ((0, cpe), (1, cpo)):
                    cr = slice(64 * lh, 64 * lh + 64)        # ctx rows
                    dr = slice(64 - 64 * lh, 128 - 64 * lh)  # denom rows
                    nc.vector.tensor_mul(
                        ctxn[hp][cr, QB * j:QB * (j + 1)], cpt[cr, :],
                        rc[dr, lh, :])

            def out_chunk(m, split_engines=False):
                ot = obp.tile([128, D], F16, tag="ob")
                for o in range(2):
                    ps = pp.tile([128, QB], F32, tag="pp")
                    for t in range(2):
                        nc.tensor.matmul(
                            ps[:], ctxn[t][:, 128 * m:128 * (m + 1)],
                            wo_sb[:, t, QB * o:QB * (o + 1)],
                            start=(t == 0), stop=(t == 1))
                    # evictions ride the DVE; on the last block the o=1
                    # half goes via ACT Copy so the two halves drain in
                    # parallel and the tail shortens.
                    osl = slice(QB * o, QB * (o + 1))
                    if split_engines and o == 1:
                        nc.scalar.activation(
                            ot[:, osl], ps[:],
                            mybir.ActivationFunctionType.Copy)
                    else:
                        nc.vector.tensor_copy(ot[:, osl], ps[:])
                    nc.sync.dma_start(
                        out.ap()[128 * m:128 * (m + 1), osl], ot[:, osl])

            def out_epilogue(cpb, et_last, c0_last):
                """Final block's out chunks, pipelined with the LAST
                attention's normalize: pair-0 halves for the pp/sp-pool
                chunks are emitted first (psum free, ctxn[0] ready), then
                per 128-col chunk the normalize (ln/exp/muls) feeds the
                pair-1 halves immediately.  The monolithic normalize
                measured 3.5us of PE idle after the last ctx matmul, which
                also dropped the HAM clock gate to half rate for the 16
                tail matmuls.  The cp-pool chunk's pair-0 halves wait for
                the chunk lns to finish reading cpb, so they are emitted
                after the last chunk's normalize.  Evictions balance
                across DVE and ACT (~5/3 halves)."""
                j = NQB - 1
                ms = list(range(4 * j, 4 * NQB))
                tiles = {}
                for k, m in enumerate(ms):
                    if k == 0:
                        a = pp.tile([128, QB], F32, tag="pp")
                        b = pp.tile([128, QB], F32, tag="pp")
                    elif k == 3:
                        cb = cp.tile([128, 2, QB], F32, tag="cp")
                        a, b = cb[:, 0], cb[:, 1]
                    else:
                        sb_ = sp.tile([128, 2, QB], F32, tag="sp")
                        a, b = sb_[:, 0], sb_[:, 1]
                    tiles[m] = (a, b)
                for m in ms[:3]:
                    for o in range(2):
                        nc.tensor.matmul(
                            tiles[m][o], ctxn[0][:, 128 * m:128 * (m + 1)],
                            wo_sb[:, 0, QB * o:QB * (o + 1)],
                            start=True, stop=False)
                # keep-alive: the serial ln->exp->mul chain idles the PE
                # ~2.9us after the pair-0 halves, dropping the HAM clock
                # gate to half rate for the pair-1 matmuls (measured
                # 584-631ns each).  Zero-weight matmuls accumulate +0.0
                # into the m12 half (numerical no-op) and keep the PE
                # streaming until the first mul chunk lands.
                # rhs = the last attention's final et tile: the RAW
                # dependency stops the Tile scheduler from hoisting these
                # ahead of the final ctx matmuls (dep-free dummies were
                # scheduled ~4us early).
                for _ in range(7):
                    nc.tensor.matmul(
                        tiles[ms[0]][0][:, 0:QB - c0_last], wz[:],
                        et_last[:, 0, c0_last:QB],
                        start=False, stop=False)
                cpe, cpo = cpb[:, 0], cpb[:, 1]
                # monolithic ln/exp (chunked ACT ops attracted batched
                # waits from later engine instructions and stalled), then
                # 256-col mul chunks so the pair-1 matmuls start as soon
                # as the first half of ctxn[1] is normalized.
                t1 = rcp.tile([128, 2, QB], F32, tag="t1")
                rc = rcp.tile([128, 2, QB], F32, tag="rc")
                nc.scalar.activation(t1[:], cpb[:], LN)
                nc.scalar.activation(rc[:], t1[:], EXP, scale=-1.0)
                for h in range(2):
                    crel = slice(256 * h, 256 * (h + 1))
                    for lh, cpt in ((0, cpe), (1, cpo)):
                        cr = slice(64 * lh, 64 * lh + 64)
                        dr = slice(64 - 64 * lh, 128 - 64 * lh)
                        nc.vector.tensor_mul(
                            ctxn[1][cr, QB * j + crel.start:
                                    QB * j + crel.stop],
                            cpt[cr, crel], rc[dr, lh, crel])
                    if h == 1:  # all cpb reads done; cp-pool chunk pair-0
                        for o in range(2):
                            nc.tensor.matmul(
                                tiles[ms[3]][o],
                                ctxn[0][:, 128 * ms[3]:128 * (ms[3] + 1)],
                                wo_sb[:, 0, QB * o:QB * (o + 1)],
                                start=True, stop=False)
                    for m in ms[2 * h:2 * h + 2]:
                        for o in range(2):
                            nc.tensor.matmul(
                                tiles[m][o],
                                ctxn[1][:, 128 * m:128 * (m + 1)],
                                wo_sb[:, 1, QB * o:QB * (o + 1)],
                                start=False, stop=True)
                # evictions AFTER the matmul loop (interleaving them on
                # DVE serialized the normalize muls behind 690ns casts):
                # ACT and DVE alternate halves; the last chunk's halves
                # DMA separately on both queues so the final transfer
                # isn't gated on both evictions.
                for k, m in enumerate(ms):
                    ot = obp.tile([128, D], F16, tag="ob")
                    for o in range(2):
                        osl = slice(QB * o, QB * (o + 1))
                        if (2 * k + o) % 2 == 0:
                            nc.vector.tensor_copy(ot[:, osl], tiles[m][o])
                        else:
                            nc.scalar.activation(
                                ot[:, osl], tiles[m][o],
                                mybir.ActivationFunctionType.Copy)
                        if k == 3:
                            eng = nc.sync if o == 0 else nc.scalar
                            eng.dma_start(
                                out.ap()[128 * m:128 * (m + 1), osl],
                                ot[:, osl])
                    if k < 3:
                        eng = nc.sync if k % 2 == 0 else nc.scalar
                        eng.dma_start(out.ap()[128 * m:128 * (m + 1), :],
                                      ot[:])

            # Interleaved schedule: per q-block j, pair-0 attention carries
            # pair-1's block-j projections as fillers; pair-1 attention
            # carries the NEXT block's pair-0 projections and v tiles.
            # Completed blocks' out-proj chunks are DEFERRED and spent as
            # fillers in the LAST block's two attentions — those are the
            # longest and have no projection work left, so without the
            # deferred chunks the PE starves behind ACT exp there.
            proj_chain(0, qT[0], wq_sb, 0)
            # wq1/wk1 issue from the gpsimd queue once the q0 eviction has
            # landed (~12.5us) — they aren't needed until ~19us, and early
            # issue would steal startup HBM bandwidth from x block 0.
            gq = wp.tile([1, 8], F16, tag="gq")
            nc.gpsimd.tensor_copy(gq[:], qT[0][0:1, QB - 8:QB])
            nc.gpsimd.dma_start(wq_sb[:, 1], wq.ap()[:, 1])
            nc.gpsimd.dma_start(wk_sb[:, 1], wk.ap()[:, 1])
            proj_chain(0, kTp[0], wk_sb, 0)
            if NQB > 1:
                # deferred issue: a dummy ACT copy reading the k0-chain
                # eviction stalls the scalar queue until ~15.5us, so the
                # x block 1 DMA doesn't compete with x block 0 + wq0/wk0
                # for startup HBM bandwidth (x1 isn't needed until ~25us).
                dgate = wp.tile([1, 8], F16, tag="dgate")
                nc.scalar.activation(dgate[:], kTp[0][0][0:1, QB - 8:QB],
                                     mybir.ActivationFunctionType.Copy)
                nc.scalar.dma_start(x_sb[:, :, QB:2 * QB],
                                    xT.ap()[:, :, QB:2 * QB])
            proj_v(0, 4)
            pending = []      # out-chunk ms whose ctxn blocks are complete
            for j in range(NQB):
                f0 = [lambda j=j: proj_chain(1, qT[1], wq_sb, j),
                      lambda j=j: proj_chain(1, kTp[1], wk_sb, j)]
                if j == NQB - 1:
                    f0 += [lambda m=m: out_chunk(m) for m in pending[:2]]
                    pending = pending[2:]
                attention_j(0, j, f0)
                if j == 0 and NQB > 2:  # x block 2 behind attn(0,0) exps
                    nc.scalar.dma_start(x_sb[:, :, 2 * QB:3 * QB],
                                        xT.ap()[:, :, 2 * QB:3 * QB])
                f1 = []
                if j + 1 < NQB:
                    f1 = [lambda j=j: proj_chain(0, qT[0], wq_sb, j + 1),
                          lambda j=j: proj_chain(0, kTp[0], wk_sb, j + 1)]
                    f1 += [lambda m=m: proj_v(m, m + 1)
                           for m in range(4 * (j + 1), 4 * (j + 2))]
                if j == 1 or j == NQB - 1:
                    # j=3 pair-1 carries 6 chunks (2/6 split with f0):
                    # measured best — the extra PE filler drains ACT's exp
                    # backlog before the last ctx matmul.
                    n_out = 4 if j == 1 else 6
                    f1 += [lambda m=m: out_chunk(m) for m in pending[:n_out]]
                    pending = pending[n_out:]
                ret = attention_j(1, j, f1, normalize=(j < NQB - 1))
                if ret is not None:
                    cpb_last, et_last, c0_last = ret
                if j == 0:  # wo + x block 3 behind attn(1,0) exps (~31us)
                    nc.scalar.dma_start(wo_sb[:], wo.ap())
                    for jj in range(3, NQB):
                        nc.scalar.dma_start(x_sb[:, :, QB * jj:QB * (jj + 1)],
                                            xT.ap()[:, :, QB * jj:QB * (jj + 1)])
                if j < NQB - 1:
                    pending += list(range(4 * j, 4 * j + 4))
            assert not pending, pending
            out_epilogue(cpb_last, et_last, c0_last)
    _pin_act_table(nc.m.arch)
    nc.compile()
    return nc


_NC = None


def _get_nc():
    global _NC
    if _NC is None:
        _NC = build_nc()
    return _NC


def make_in_maps(x, Wq, Wk, Wv, Wo):
    x = np.asarray(x, np.float32)
    Wq, Wk, Wv, Wo = (np.asarray(w, np.float32) for w in (Wq, Wk, Wv, Wo))
    in_maps = []
    for c in range(8):
        b, g = c // 4, c % 4
        sl = slice(DC * g, DC * (g + 1))
        # wq/wk host layout [128, 2, NDT, 128]: [p, hp, t, c] =
        # W[t*128+p, 256g + 128hp + c] so each pair half is one contiguous
        # DMA and chains slice [:, hp, t, :].
        wq_l = (Wq[:, sl].astype(np.float16).reshape(NDT, 128, 2, 128)
                .transpose(1, 2, 0, 3))
        wk_l = (Wk[:, sl].astype(np.float16).reshape(NDT, 128, 2, 128)
                .transpose(1, 2, 0, 3))
        in_maps.append({
            "xT": np.ascontiguousarray(
                x[b].T.astype(np.float16).reshape(NDT, 128, S)
                .transpose(1, 0, 2)),
            "wq": np.ascontiguousarray(wq_l),
            "wk": np.ascontiguousarray(wk_l),
            "wv": np.ascontiguousarray(
                Wv[:, sl].astype(np.float16).reshape(NDT, 128, DC)
                .transpose(1, 0, 2)),
            "wo": np.ascontiguousarray(
                Wo[sl, :].astype(np.float16).reshape(2, 128, D)
                .transpose(1, 0, 2)),
        })
    return in_maps


def kernel(x, Wq, Wk, Wv, Wo, bo, _trace=False, _trace_cores=None):
    nc = _get_nc()
    in_maps = make_in_maps(x, Wq, Wk, Wv, Wo)
    res = bass_utils.run_bass_kernel_spmd(
        nc, in_maps, core_ids=list(range(8)), trace=_trace,
        trace_cores=_trace_cores)
    bo = np.asarray(bo, np.float32)
    out = np.empty((B, S, D), np.float32)
    for b in range(B):
        acc = res.results[4 * b]["out_p"].astype(np.float32)
        for g in range(1, 4):
            acc += res.results[4 * b + g]["out_p"].astype(np.float32)
        out[b] = acc + bo
    kernel.last_results = res
    return out

